# revision 22
# baseline (speedup 1.0000x reference)
"""GCN-VAE (2x GCNConv -> concat -> 2x GCNConv -> inner-product decode) on 8 trn2 cores.

Math (reference):
  S = D^-1/2 (A + I) D^-1/2   (sparse, built from edge_index; D = in-degree of `row`)
  hc = S @ (x @ [W1|W2]) + [b1|b2]          # [N, 256]
  mz = S @ (hc @ [Wmu|Wlv]) + [bmu|blv]     # [N, 128] = [mu | logvar]
  adj = sigmoid(mu @ mu.T)                  # [N, N]

Device strategy (node/row sharding across 8 cores):
  * S is materialized dense on host (0.8% nnz -> dense PE matmul beats scatter),
    passed TRANSPOSED per core: ST_i = S[rows_i, :].T  [N, 1024] bf16.
  * Feature-major ("transposed") on-device layouts avoid all on-chip transposes:
      G0    = x @ Wc1             natural [j, 256]  (lhsT for the S-matmul)
      hcT_i = G0.T @ ST_i + b     [256, 1024]  (local rows only)
      G1_i  = hc_i @ Wc2          natural [1024, 128], computed locally
              -> AllGather G1 (2MB) -> G1 full
      mzT_i = G1.T @ ST_i + b     [128, 1024]
              -> AllGather mu rows only (1MB) -> muT full
      adj_i = sigmoid(mu_i @ mu.T)   row-shard [1024, N], written bf16
  * bf16 inputs / fp32 PSUM accumulation everywhere; adj output bf16 (host
    upcasts; sigmoid output ~0.5 so bf16 abs err ~1e-3).
  * First half of ST (columns 0:512 of each core's shard) stays resident in
    SBUF after stage B so stage D only re-streams the second half.
"""

import os
import sys

sys.path.insert(0, "/opt/trn_rl_repo")

import numpy as np
import ml_dtypes

N = 8192
NCORES = 8
ROWS = N // NCORES  # 1024
IN_DIM = 256
HC_DIM = 256  # 2 * HID1
MZ_DIM = 128  # 2 * HID2
BF16 = ml_dtypes.bfloat16

_cache: dict = {}


def _build_program():
    import concourse.bacc as bacc
    import concourse.mybir as mybir
    import concourse.tile as tile

    f32 = mybir.dt.float32
    bf16 = mybir.dt.bfloat16

    nc = bacc.Bacc("TRN2", target_bir_lowering=False, debug=False, num_devices=NCORES)

    st_d = nc.dram_tensor("st", [N, ROWS], bf16, kind="ExternalInput")
    xT_d = nc.dram_tensor("xT", [2, 128, N], bf16, kind="ExternalInput")
    wc1_d = nc.dram_tensor("wc1", [2, 128, HC_DIM], bf16, kind="ExternalInput")
    wc2_d = nc.dram_tensor("wc2", [2, 128, MZ_DIM], bf16, kind="ExternalInput")
    bc1_d = nc.dram_tensor("bc1", [128, 2], f32, kind="ExternalInput")
    bc2_d = nc.dram_tensor("bc2", [128, 1], f32, kind="ExternalInput")
    adj_d = nc.dram_tensor("adj", [ROWS, N], bf16, kind="ExternalOutput")
    mz_d = nc.dram_tensor("mz", [128, ROWS], f32, kind="ExternalOutput")

    with tile.TileContext(nc) as tc:
        with (
            tc.tile_pool(name="const", bufs=1) as constp,
            tc.tile_pool(name="big", bufs=1) as bigp,
            tc.tile_pool(name="stream", bufs=1) as streamp,
            tc.tile_pool(name="psum", bufs=1, space="PSUM") as psump,
            tc.tile_pool(name="outp", bufs=1) as outp,
            tc.tile_pool(name="dram", bufs=1, space="DRAM") as dramp,
        ):
            # ------- constants -------
            wc1_sb = constp.tile([128, 2, HC_DIM], bf16, name="wc1_sb")
            wc2_sb = constp.tile([128, 2, MZ_DIM], bf16, name="wc2_sb")
            bc1_sb = constp.tile([128, 2], f32, name="bc1_sb")
            bc2_sb = constp.tile([128, 1], f32, name="bc2_sb")
            for kc in range(2):
                nc.sync.dma_start(out=wc1_sb[:, kc, :], in_=wc1_d[kc])
                nc.sync.dma_start(out=wc2_sb[:, kc, :], in_=wc2_d[kc])
            nc.sync.dma_start(out=bc1_sb[:], in_=bc1_d[:])
            nc.sync.dma_start(out=bc2_sb[:], in_=bc2_d[:])

            # xT shares its slot with STres (xT dead after stage A).
            xT_sb = bigp.tile([128, 2, N], bf16, name="xT_sb", tag="bigslot")
            for kc in range(2):
                nc.gpsimd.dma_start(out=xT_sb[:, kc, :], in_=xT_d[kc])

            # ------- stage A: G0 = x @ Wc1  -> [128(j_lo), 64(jc), 256] bf16 -------
            G0_sb = bigp.tile([128, 64, HC_DIM], bf16, name="G0_sb", tag="g0slot")
            for jc in range(64):
                g0p = psump.tile([128, 512], f32, name="g0p", tag="pp", bufs=4)
                for kc in range(2):
                    nc.tensor.matmul(
                        g0p[:, 0:HC_DIM],
                        xT_sb[:, kc, jc * 128 : (jc + 1) * 128],
                        wc1_sb[:, kc, :],
                        start=(kc == 0),
                        stop=(kc == 1),
                    )
                nc.vector.tensor_copy(G0_sb[:, jc, :], g0p[:, 0:HC_DIM])

            # ------- stage B: hcT_i = G0.T @ ST_i + bc1 -> [128, 2(h), 1024] bf16 ---
            # BOTH halves of ST land in resident SBUF tiles (reused in stage D,
            # which then needs no DMA at all). STB shares xT's slot (xT is dead
            # after stage A).
            STA = bigp.tile([128, 64, 512], bf16, name="STA")
            STB = bigp.tile([128, 64, 512], bf16, name="STB", tag="bigslot")
            hcT_sb = bigp.tile([128, 2, ROWS], bf16, name="hcT_sb")

            def emit_stage_b(ncc):
                hp0 = psump.tile([128, 512], f32, name="hp0", tag="pp", bufs=4)
                hp1 = psump.tile([128, 512], f32, name="hp1", tag="pp", bufs=4)
                for jc in range(64):
                    stt = (STA if ncc == 0 else STB)[:, jc, :]
                    nc.sync.dma_start(
                        out=stt,
                        in_=st_d[jc * 128 : (jc + 1) * 128, ncc * 512 : (ncc + 1) * 512],
                    )
                    nc.tensor.matmul(
                        hp0[:], G0_sb[:, jc, 0:128], stt,
                        start=(jc == 0), stop=(jc == 63),
                    )
                    nc.tensor.matmul(
                        hp1[:], G0_sb[:, jc, 128:256], stt,
                        start=(jc == 0), stop=(jc == 63),
                    )
                nc.vector.tensor_scalar_add(
                    hcT_sb[:, 0, ncc * 512 : (ncc + 1) * 512], hp0[:], bc1_sb[:, 0:1]
                )
                nc.vector.tensor_scalar_add(
                    hcT_sb[:, 1, ncc * 512 : (ncc + 1) * 512], hp1[:], bc1_sb[:, 1:2]
                )

            # ------- stage C (local): G1_i = hc_i @ Wc2 -> [1024, 128] ----------
            # Interleaved with stage B: the hcT half from B(ncc) feeds C(half=ncc)
            # immediately, so each G1 AllGather overlaps the other stage-B half /
            # stage-D work.
            G1loc = bigp.tile([128, 8, MZ_DIM], bf16, name="G1loc")
            G1_all = bigp.tile([128, 64, MZ_DIM], bf16, name="G1_all")
            g1_gaths = []
            for half in range(2):
                emit_stage_b(half)
                for jl in range(half * 4, half * 4 + 4):
                    g1p = psump.tile([128, 512], f32, name="g1p", tag="pp", bufs=4)
                    for cc in range(2):
                        nc.tensor.matmul(
                            g1p[:, 0:MZ_DIM],
                            hcT_sb[:, cc, jl * 128 : (jl + 1) * 128],
                            wc2_sb[:, cc, :],
                            start=(cc == 0),
                            stop=(cc == 1),
                        )
                    nc.vector.tensor_copy(G1loc[:, jl, :], g1p[:, 0:MZ_DIM])
                g1_bounce = dramp.tile(
                    [512, MZ_DIM], bf16, name=f"g1_bounce{half}", tag=f"g1b{half}"
                )
                for jl in range(4):
                    nc.gpsimd.dma_start(
                        out=g1_bounce[jl * 128 : (jl + 1) * 128, :],
                        in_=G1loc[:, half * 4 + jl, :],
                    )
                g1_gath = dramp.tile(
                    [NCORES, 512, MZ_DIM],
                    bf16,
                    name=f"g1_gath{half}",
                    tag=f"g1g{half}",
                    addr_space="Shared",
                )
                nc.gpsimd.collective_compute(
                    "AllGather",
                    mybir.AluOpType.bypass,
                    replica_groups=[list(range(NCORES))],
                    ins=[g1_bounce.opt()],
                    outs=[g1_gath.opt()],
                )
                g1_gaths.append(g1_gath)
                for s in range(NCORES):
                    for jl in range(4):
                        nc.gpsimd.dma_start(
                            out=G1_all[:, s * 8 + half * 4 + jl, :],
                            in_=g1_gath[s, jl * 128 : (jl + 1) * 128, :],
                        )

            # ------- stage D: mzT_i = G1.T @ ST_i + bc2 -> [128, 2, 512] -------
            mzT_f32 = bigp.tile([128, 2, 512], f32, name="mzT_f32")
            mzT_bf = bigp.tile([128, 2, 512], bf16, name="mzT_bf")
            # Accumulate half-A chunks (gathered first) before half-B chunks so
            # the B-half AllGather overlaps the A-half matmuls.
            chunk_order = [s * 8 + jl for jl in range(4) for s in range(NCORES)] + [
                s * 8 + 4 + jl for jl in range(4) for s in range(NCORES)
            ]
            # Both ncc chains interleaved chunk-by-chunk: all half-A chunks (both
            # chains) are emitted before any half-B chunk, so the PE never
            # head-of-line blocks on the half-B AllGather while half-A work is
            # ready.
            mp0 = psump.tile([128, 512], f32, name="mp0", tag="pp", bufs=4)
            mp1 = psump.tile([128, 512], f32, name="mp1", tag="pp", bufs=4)
            for k, jc in enumerate(chunk_order):
                nc.tensor.matmul(
                    mp0[:], G1_all[:, jc, :], STA[:, jc, :],
                    start=(k == 0), stop=(k == 63),
                )
                nc.tensor.matmul(
                    mp1[:], G1_all[:, jc, :], STB[:, jc, :],
                    start=(k == 0), stop=(k == 63),
                )
            for ncc, mp in enumerate((mp0, mp1)):
                nc.vector.tensor_scalar_add(mzT_f32[:, ncc, :], mp[:], bc2_sb[:, 0:1])
                nc.vector.tensor_copy(mzT_bf[:, ncc, :], mzT_f32[:, ncc, :])
                nc.sync.dma_start(
                    out=mz_d[:, ncc * 512 : (ncc + 1) * 512], in_=mzT_f32[:, ncc, :]
                )

            # ------- AllGather mu rows of mzT, split by local row half (ncc) ----
            # The ncc=0 gather overlaps the ncc=1 stage-D chain; decode of the
            # gathered columns starts as soon as its half arrives.
            # both halves live in one tile that reuses G0's slot (G0 is dead)
            muT_all = bigp.tile([64, 2, NCORES, 512], bf16, name="muT_all", tag="g0slot")
            for ncc in range(2):
                mu_bounce = dramp.tile(
                    [64, 512], bf16, name=f"mu_bounce{ncc}", tag=f"mub{ncc}"
                )
                nc.scalar.dma_start(out=mu_bounce[:], in_=mzT_bf[0:64, ncc, :])
                mu_gath = dramp.tile(
                    [NCORES, 64, 512],
                    bf16,
                    name=f"mu_gath{ncc}",
                    tag=f"mug{ncc}",
                    addr_space="Shared",
                )
                nc.gpsimd.collective_compute(
                    "AllGather",
                    mybir.AluOpType.bypass,
                    replica_groups=[list(range(NCORES))],
                    ins=[mu_bounce.opt()],
                    outs=[mu_gath.opt()],
                )
                for s in range(NCORES):
                    nc.gpsimd.dma_start(out=muT_all[:, ncc, s, :], in_=mu_gath[s])

            # ------- decode: adj_i = sigmoid(mu_i @ mu.T), bf16 out -------
            # Global column block (s, half) covers adj cols s*1024+half*512 ..+512.
            for half in range(2):
                muT = muT_all[:, half]
                for rc in range(8):
                    ncc, off = divmod(rc * 128, 512)
                    lhsT = mzT_bf[0:64, ncc, off : off + 128]
                    for s in range(NCORES):
                        dp = psump.tile([128, 512], f32, name="dp", tag="pp", bufs=4)
                        nc.tensor.matmul(
                            dp[:], lhsT, muT[:, s, :], start=True, stop=True
                        )
                        adj_sb = outp.tile(
                            [128, 512], bf16, name="adj_sb", tag="adj_sb", bufs=4
                        )
                        nc.scalar.activation(
                            adj_sb[:], dp[:], mybir.ActivationFunctionType.Sigmoid
                        )
                        nc.sync.dma_start(
                            out=adj_d[
                                rc * 128 : (rc + 1) * 128,
                                s * 1024 + half * 512 : s * 1024 + half * 512 + 512,
                            ],
                            in_=adj_sb[:],
                        )

    nc.compile()
    return nc


def _get_program():
    if "nc" not in _cache:
        _cache["nc"] = _build_program()
    return _cache["nc"]


def kernel(**inputs):
    x = np.asarray(inputs["x"], dtype=np.float32)
    ei = np.asarray(inputs["edge_index"]).astype(np.int64)
    W1 = np.asarray(inputs["W1"], dtype=np.float32)
    b1 = np.asarray(inputs["b1"], dtype=np.float32)
    W2 = np.asarray(inputs["W2"], dtype=np.float32)
    b2 = np.asarray(inputs["b2"], dtype=np.float32)
    Wmu = np.asarray(inputs["Wmu"], dtype=np.float32)
    bmu = np.asarray(inputs["bmu"], dtype=np.float32)
    Wlv = np.asarray(inputs["Wlv"], dtype=np.float32)
    blv = np.asarray(inputs["blv"], dtype=np.float32)

    # --- normalized adjacency, transposed, dense ---
    loop = np.arange(N, dtype=np.int64)
    row = np.concatenate([ei[0], loop])
    col = np.concatenate([ei[1], loop])
    deg = np.bincount(row, minlength=N).astype(np.float32)
    dinv = np.where(deg > 0, 1.0 / np.sqrt(deg), 0.0).astype(np.float32)
    norm = (dinv[row] * dinv[col]).astype(np.float32)
    ST = np.zeros((N, N), dtype=np.float32)
    np.add.at(ST, (col, row), norm)  # ST[c, r] = S[r, c]
    ST_bf = ST.astype(BF16)

    Wc1 = np.concatenate([W1, W2], axis=1)  # [256, 256]
    Wc2 = np.concatenate([Wmu, Wlv], axis=1)  # [256, 128]
    bc1 = np.ascontiguousarray(
        np.concatenate([b1, b2]).reshape(2, 128).T.astype(np.float32)
    )
    bc2 = np.concatenate([bmu, blv]).reshape(128, 1).astype(np.float32)
    xT = np.ascontiguousarray(x.T).astype(BF16).reshape(2, 128, N)
    wc1 = Wc1.astype(BF16).reshape(2, 128, HC_DIM)
    wc2 = Wc2.astype(BF16).reshape(2, 128, MZ_DIM)

    in_maps = []
    for i in range(NCORES):
        in_maps.append(
            {
                "st": np.ascontiguousarray(ST_bf[:, i * ROWS : (i + 1) * ROWS]),
                "xT": xT,
                "wc1": wc1,
                "wc2": wc2,
                "bc1": bc1,
                "bc2": bc2,
            }
        )

    from concourse.bass_utils import run_bass_kernel_spmd

    nc = _get_program()
    trace = os.environ.get("KERNEL_TRACE", "0") == "1"
    res = run_bass_kernel_spmd(nc, in_maps, core_ids=list(range(NCORES)), trace=trace)
    if trace and res.exec_time_ns is not None:
        print(f"HW exec time: {res.exec_time_ns} ns")
        _cache["exec_time_ns"] = res.exec_time_ns
    rs = res.results

    adj = np.concatenate(
        [rs[i]["adj"].astype(np.float32) for i in range(NCORES)], axis=0
    )
    mzs = [rs[i]["mz"] for i in range(NCORES)]  # [128, 1024] each, f32
    mu = np.concatenate([m[:64].T for m in mzs], axis=0)
    logvar = np.concatenate([m[64:].T for m in mzs], axis=0)
    return adj, mu, logvar


# revision 27
# speedup vs baseline: 1.0905x; 1.0905x over previous
"""GCN-VAE (2x GCNConv -> concat -> 2x GCNConv -> inner-product decode) on 8 trn2 cores.

Math (reference):
  S = D^-1/2 (A + I) D^-1/2   (sparse, built from edge_index; D = in-degree of `row`)
  hc = S @ (x @ [W1|W2]) + [b1|b2]          # [N, 256]
  mz = S @ (hc @ [Wmu|Wlv]) + [bmu|blv]     # [N, 128] = [mu | logvar]
  adj = sigmoid(mu @ mu.T)                  # [N, N]

Device strategy (node/row sharding across 8 cores):
  * S is materialized dense on host (0.8% nnz -> dense PE matmul beats scatter),
    passed TRANSPOSED per core: ST_i = S[rows_i, :].T  [N, 1024] bf16.
  * Feature-major ("transposed") on-device layouts avoid all on-chip transposes:
      G0    = x @ Wc1             natural [j, 256]  (lhsT for the S-matmul)
      hcT_i = G0.T @ ST_i + b     [256, 1024]  (local rows only)
      G1_i  = hc_i @ Wc2          natural [1024, 128], computed locally
              -> AllGather G1 (2MB) -> G1 full
      mzT_i = G1.T @ ST_i + b     [128, 1024]
              -> AllGather mu rows only (1MB) -> muT full
      adj_i = sigmoid(mu_i @ mu.T)   row-shard [1024, N], written bf16
  * bf16 inputs / fp32 PSUM accumulation everywhere; adj output bf16 (host
    upcasts; sigmoid output ~0.5 so bf16 abs err ~1e-3).
  * First half of ST (columns 0:512 of each core's shard) stays resident in
    SBUF after stage B so stage D only re-streams the second half.
"""

import os
import sys

sys.path.insert(0, "/opt/trn_rl_repo")

import numpy as np
import ml_dtypes

N = 8192
NCORES = 8
ROWS = N // NCORES  # 1024
IN_DIM = 256
HC_DIM = 256  # 2 * HID1
MZ_DIM = 128  # 2 * HID2
BF16 = ml_dtypes.bfloat16

_cache: dict = {}


def _build_program():
    import concourse.bacc as bacc
    import concourse.mybir as mybir
    import concourse.tile as tile

    f32 = mybir.dt.float32
    bf16 = mybir.dt.bfloat16

    nc = bacc.Bacc("TRN2", target_bir_lowering=False, debug=False, num_devices=NCORES)

    st_d = nc.dram_tensor("st", [N, ROWS], bf16, kind="ExternalInput")
    xT_d = nc.dram_tensor("xT", [2, 128, N], bf16, kind="ExternalInput")
    wc1_d = nc.dram_tensor("wc1", [2, 128, HC_DIM], bf16, kind="ExternalInput")
    wc2_d = nc.dram_tensor("wc2", [2, 128, MZ_DIM], bf16, kind="ExternalInput")
    bc1_d = nc.dram_tensor("bc1", [128, 2], f32, kind="ExternalInput")
    bc2_d = nc.dram_tensor("bc2", [128, 1], f32, kind="ExternalInput")
    adj_d = nc.dram_tensor("adj", [ROWS, N], bf16, kind="ExternalOutput")
    mz_d = nc.dram_tensor("mz", [128, ROWS], f32, kind="ExternalOutput")

    with tile.TileContext(nc) as tc:
        with (
            tc.tile_pool(name="const", bufs=1) as constp,
            tc.tile_pool(name="big", bufs=1) as bigp,
            tc.tile_pool(name="stream", bufs=1) as streamp,
            tc.tile_pool(name="psum", bufs=1, space="PSUM") as psump,
            tc.tile_pool(name="outp", bufs=1) as outp,
            tc.tile_pool(name="dram", bufs=1, space="DRAM") as dramp,
        ):
            # ------- constants -------
            wc1_sb = constp.tile([128, 2, HC_DIM], bf16, name="wc1_sb")
            wc2_sb = constp.tile([128, 2, MZ_DIM], bf16, name="wc2_sb")
            bc1_sb = constp.tile([128, 2], f32, name="bc1_sb")
            bc2_sb = constp.tile([128, 1], f32, name="bc2_sb")
            for kc in range(2):
                nc.sync.dma_start(out=wc1_sb[:, kc, :], in_=wc1_d[kc])
                nc.sync.dma_start(out=wc2_sb[:, kc, :], in_=wc2_d[kc])
            nc.sync.dma_start(out=bc1_sb[:], in_=bc1_d[:])
            nc.sync.dma_start(out=bc2_sb[:], in_=bc2_d[:])

            # xT shares its slot with STres (xT dead after stage A).
            xT_sb = bigp.tile([128, 2, N], bf16, name="xT_sb", tag="bigslot")
            for kc in range(2):
                nc.gpsimd.dma_start(out=xT_sb[:, kc, :], in_=xT_d[kc])

            # ------- stage A: G0 = x @ Wc1  -> [128(j_lo), 32(jc pair), 512] bf16 --
            # Two j-chunks per PSUM tile -> half the PSUM->SBUF copies, and the
            # copies alternate DVE / ACT (both otherwise idle here).
            G0_sb = bigp.tile([128, 32, 512], bf16, name="G0_sb", tag="g0slot")
            for jcp in range(32):
                g0p = psump.tile([128, 512], f32, name="g0p", tag="pp", bufs=4)
                for sub in range(2):
                    jc = jcp * 2 + sub
                    for kc in range(2):
                        nc.tensor.matmul(
                            g0p[:, sub * 256 : sub * 256 + HC_DIM],
                            xT_sb[:, kc, jc * 128 : (jc + 1) * 128],
                            wc1_sb[:, kc, :],
                            start=(kc == 0),
                            stop=(kc == 1),
                        )
                if jcp % 2 == 0:
                    nc.vector.tensor_copy(G0_sb[:, jcp, :], g0p[:])
                else:
                    nc.scalar.activation(
                        G0_sb[:, jcp, :], g0p[:], mybir.ActivationFunctionType.Copy
                    )

            # ------- stage B: hcT_i = G0.T @ ST_i + bc1 -> [128, 2(h), 1024] bf16 ---
            # BOTH halves of ST land in resident SBUF tiles (reused in stage D,
            # which then needs no DMA at all). STB shares xT's slot (xT is dead
            # after stage A).
            STA = bigp.tile([128, 64, 512], bf16, name="STA")
            STB = bigp.tile([128, 64, 512], bf16, name="STB", tag="bigslot")
            hcT_sb = bigp.tile([128, 2, ROWS], bf16, name="hcT_sb")

            def emit_stage_b(ncc):
                hp0 = psump.tile([128, 512], f32, name="hp0", tag="pp", bufs=4)
                hp1 = psump.tile([128, 512], f32, name="hp1", tag="pp", bufs=4)
                for jc in range(64):
                    stt = (STA if ncc == 0 else STB)[:, jc, :]
                    nc.sync.dma_start(
                        out=stt,
                        in_=st_d[jc * 128 : (jc + 1) * 128, ncc * 512 : (ncc + 1) * 512],
                    )
                    g0base = (jc % 2) * 256
                    nc.tensor.matmul(
                        hp0[:], G0_sb[:, jc // 2, g0base : g0base + 128], stt,
                        start=(jc == 0), stop=(jc == 63),
                    )
                    nc.tensor.matmul(
                        hp1[:], G0_sb[:, jc // 2, g0base + 128 : g0base + 256], stt,
                        start=(jc == 0), stop=(jc == 63),
                    )
                nc.vector.tensor_scalar_add(
                    hcT_sb[:, 0, ncc * 512 : (ncc + 1) * 512], hp0[:], bc1_sb[:, 0:1]
                )
                nc.vector.tensor_scalar_add(
                    hcT_sb[:, 1, ncc * 512 : (ncc + 1) * 512], hp1[:], bc1_sb[:, 1:2]
                )

            # ------- stage C (local): G1_i = hc_i @ Wc2 -> [1024, 128] ----------
            # Interleaved with stage B: the hcT half from B(ncc) feeds C(half=ncc)
            # immediately, so each G1 AllGather overlaps the other stage-B half /
            # stage-D work.
            G1loc = bigp.tile([128, 8, MZ_DIM], bf16, name="G1loc")
            G1_all = bigp.tile([128, 64, MZ_DIM], bf16, name="G1_all")
            g1_gaths = []
            for half in range(2):
                emit_stage_b(half)
                for jl in range(half * 4, half * 4 + 4):
                    g1p = psump.tile([128, 512], f32, name="g1p", tag="pp", bufs=4)
                    for cc in range(2):
                        nc.tensor.matmul(
                            g1p[:, 0:MZ_DIM],
                            hcT_sb[:, cc, jl * 128 : (jl + 1) * 128],
                            wc2_sb[:, cc, :],
                            start=(cc == 0),
                            stop=(cc == 1),
                        )
                    nc.vector.tensor_copy(G1loc[:, jl, :], g1p[:, 0:MZ_DIM])
                g1_bounce = dramp.tile(
                    [512, MZ_DIM], bf16, name=f"g1_bounce{half}", tag=f"g1b{half}"
                )
                for jl in range(4):
                    nc.gpsimd.dma_start(
                        out=g1_bounce[jl * 128 : (jl + 1) * 128, :],
                        in_=G1loc[:, half * 4 + jl, :],
                    )
                g1_gath = dramp.tile(
                    [NCORES, 512, MZ_DIM],
                    bf16,
                    name=f"g1_gath{half}",
                    tag=f"g1g{half}",
                    addr_space="Shared",
                )
                nc.gpsimd.collective_compute(
                    "AllGather",
                    mybir.AluOpType.bypass,
                    replica_groups=[list(range(NCORES))],
                    ins=[g1_bounce.opt()],
                    outs=[g1_gath.opt()],
                )
                g1_gaths.append(g1_gath)
                for s in range(NCORES):
                    for jl in range(4):
                        nc.gpsimd.dma_start(
                            out=G1_all[:, s * 8 + half * 4 + jl, :],
                            in_=g1_gath[s, jl * 128 : (jl + 1) * 128, :],
                        )

            # ------- stage D: mzT_i = G1.T @ ST_i + bc2 -> [128, 2, 512] -------
            mzT_f32 = bigp.tile([128, 2, 512], f32, name="mzT_f32")
            mzT_bf = bigp.tile([128, 2, 512], bf16, name="mzT_bf")
            # Accumulate half-A chunks (gathered first) before half-B chunks so
            # the B-half AllGather overlaps the A-half matmuls.
            chunk_order = [s * 8 + jl for jl in range(4) for s in range(NCORES)] + [
                s * 8 + 4 + jl for jl in range(4) for s in range(NCORES)
            ]
            # Both ncc chains interleaved chunk-by-chunk: all half-A chunks (both
            # chains) are emitted before any half-B chunk, so the PE never
            # head-of-line blocks on the half-B AllGather while half-A work is
            # ready.
            mp0 = psump.tile([128, 512], f32, name="mp0", tag="pp", bufs=4)
            mp1 = psump.tile([128, 512], f32, name="mp1", tag="pp", bufs=4)
            muT_all = bigp.tile([64, 2, NCORES, 512], bf16, name="muT_all", tag="g0slot")

            def emit_mu_gather(ncc, mp):
                # mzT epilogue for one local-row half + its AllGather launch
                nc.vector.tensor_scalar_add(mzT_f32[:, ncc, :], mp[:], bc2_sb[:, 0:1])
                nc.vector.tensor_copy(mzT_bf[:, ncc, :], mzT_f32[:, ncc, :])
                nc.sync.dma_start(
                    out=mz_d[:, ncc * 512 : (ncc + 1) * 512], in_=mzT_f32[:, ncc, :]
                )
                mu_bounce = dramp.tile(
                    [64, 512], bf16, name=f"mu_bounce{ncc}", tag=f"mub{ncc}"
                )
                nc.scalar.dma_start(out=mu_bounce[:], in_=mzT_bf[0:64, ncc, :])
                mu_gath = dramp.tile(
                    [NCORES, 64, 512],
                    bf16,
                    name=f"mu_gath{ncc}",
                    tag=f"mug{ncc}",
                    addr_space="Shared",
                )
                nc.gpsimd.collective_compute(
                    "AllGather",
                    mybir.AluOpType.bypass,
                    replica_groups=[list(range(NCORES))],
                    ins=[mu_bounce.opt()],
                    outs=[mu_gath.opt()],
                )
                for s in range(NCORES):
                    nc.gpsimd.dma_start(out=muT_all[:, ncc, s, :], in_=mu_gath[s])

            # Half-A chunks interleaved across both chains; then mp0 finishes its
            # half-B chunks FIRST so the ncc=0 mu AllGather launches while mp1's
            # remaining matmuls still occupy the PE.
            half_a, half_b = chunk_order[:32], chunk_order[32:]
            for k, jc in enumerate(half_a):
                nc.tensor.matmul(
                    mp0[:], G1_all[:, jc, :], STA[:, jc, :],
                    start=(k == 0), stop=False,
                )
                nc.tensor.matmul(
                    mp1[:], G1_all[:, jc, :], STB[:, jc, :],
                    start=(k == 0), stop=False,
                )
            for k, jc in enumerate(half_b):
                nc.tensor.matmul(
                    mp0[:], G1_all[:, jc, :], STA[:, jc, :],
                    start=False, stop=(k == 31),
                )
            emit_mu_gather(0, mp0)
            for k, jc in enumerate(half_b):
                nc.tensor.matmul(
                    mp1[:], G1_all[:, jc, :], STB[:, jc, :],
                    start=False, stop=(k == 31),
                )
            emit_mu_gather(1, mp1)

            # ------- decode: adj_i = sigmoid(mu_i @ mu.T), bf16 out -------
            # Global column block (s, half) covers adj cols s*1024+half*512 ..+512.
            for half in range(2):
                muT = muT_all[:, half]
                for rc in range(8):
                    ncc, off = divmod(rc * 128, 512)
                    lhsT = mzT_bf[0:64, ncc, off : off + 128]
                    for s in range(NCORES):
                        dp = psump.tile([128, 512], f32, name="dp", tag="pp", bufs=4)
                        nc.tensor.matmul(
                            dp[:], lhsT, muT[:, s, :], start=True, stop=True
                        )
                        adj_sb = outp.tile(
                            [128, 512], bf16, name="adj_sb", tag="adj_sb", bufs=8
                        )
                        nc.scalar.activation(
                            adj_sb[:], dp[:], mybir.ActivationFunctionType.Sigmoid
                        )
                        nc.sync.dma_start(
                            out=adj_d[
                                rc * 128 : (rc + 1) * 128,
                                s * 1024 + half * 512 : s * 1024 + half * 512 + 512,
                            ],
                            in_=adj_sb[:],
                        )

    nc.compile()
    return nc


def _get_program():
    if "nc" not in _cache:
        _cache["nc"] = _build_program()
    return _cache["nc"]


def kernel(**inputs):
    x = np.asarray(inputs["x"], dtype=np.float32)
    ei = np.asarray(inputs["edge_index"]).astype(np.int64)
    W1 = np.asarray(inputs["W1"], dtype=np.float32)
    b1 = np.asarray(inputs["b1"], dtype=np.float32)
    W2 = np.asarray(inputs["W2"], dtype=np.float32)
    b2 = np.asarray(inputs["b2"], dtype=np.float32)
    Wmu = np.asarray(inputs["Wmu"], dtype=np.float32)
    bmu = np.asarray(inputs["bmu"], dtype=np.float32)
    Wlv = np.asarray(inputs["Wlv"], dtype=np.float32)
    blv = np.asarray(inputs["blv"], dtype=np.float32)

    # --- normalized adjacency, transposed, dense ---
    loop = np.arange(N, dtype=np.int64)
    row = np.concatenate([ei[0], loop])
    col = np.concatenate([ei[1], loop])
    deg = np.bincount(row, minlength=N).astype(np.float32)
    dinv = np.where(deg > 0, 1.0 / np.sqrt(deg), 0.0).astype(np.float32)
    norm = (dinv[row] * dinv[col]).astype(np.float32)
    ST = np.zeros((N, N), dtype=np.float32)
    np.add.at(ST, (col, row), norm)  # ST[c, r] = S[r, c]
    ST_bf = ST.astype(BF16)

    Wc1 = np.concatenate([W1, W2], axis=1)  # [256, 256]
    Wc2 = np.concatenate([Wmu, Wlv], axis=1)  # [256, 128]
    bc1 = np.ascontiguousarray(
        np.concatenate([b1, b2]).reshape(2, 128).T.astype(np.float32)
    )
    bc2 = np.concatenate([bmu, blv]).reshape(128, 1).astype(np.float32)
    xT = np.ascontiguousarray(x.T).astype(BF16).reshape(2, 128, N)
    wc1 = Wc1.astype(BF16).reshape(2, 128, HC_DIM)
    wc2 = Wc2.astype(BF16).reshape(2, 128, MZ_DIM)

    in_maps = []
    for i in range(NCORES):
        in_maps.append(
            {
                "st": np.ascontiguousarray(ST_bf[:, i * ROWS : (i + 1) * ROWS]),
                "xT": xT,
                "wc1": wc1,
                "wc2": wc2,
                "bc1": bc1,
                "bc2": bc2,
            }
        )

    from concourse.bass_utils import run_bass_kernel_spmd

    nc = _get_program()
    trace = os.environ.get("KERNEL_TRACE", "0") == "1"
    res = run_bass_kernel_spmd(nc, in_maps, core_ids=list(range(NCORES)), trace=trace)
    if trace and res.exec_time_ns is not None:
        print(f"HW exec time: {res.exec_time_ns} ns")
        _cache["exec_time_ns"] = res.exec_time_ns
    rs = res.results

    adj = np.concatenate(
        [rs[i]["adj"].astype(np.float32) for i in range(NCORES)], axis=0
    )
    mzs = [rs[i]["mz"] for i in range(NCORES)]  # [128, 1024] each, f32
    mu = np.concatenate([m[:64].T for m in mzs], axis=0)
    logvar = np.concatenate([m[64:].T for m in mzs], axis=0)
    return adj, mu, logvar
